# revision 55
# baseline (speedup 1.0000x reference)
"""RBF (Gaussian) kernel matrix on 8 TRN2 NeuronCores.

out[i, j] = exp(-gamma * ||x_i - y_j||^2),  x: [8192, 64], y: [8192, 64].

Strategy: shard rows of x across 8 cores (each computes a [1024, 8192]
tile), replicate y.  The squared distance is produced directly by matmul
via augmented vectors:

    u_i = [-2*x_i, |x_i|^2, 1]   (67 live rows)
    v_j = [   y_j,       1, |y_j|^2]

so  u_i . v_j = |x_i|^2 + |y_j|^2 - 2 x_i.y_j = dist2[i, j].  PSUM then
holds dist2 directly and one ScalarE activation computes
exp(-gamma * dist2) per [128, 2048] tile — no vector-engine work at all.

The matmul runs at K=128 (measured ~1.5x faster per 512-column tile
than K=67), so the operands occupy 128 partitions with rows 67-127
zero.  DMA loads of <128-partition tiles can degenerate to serial
single-engine descriptors (~27 GB/s) while 64- and 128-partition loads
spray across all 16 SDMA engines (~400 GB/s), hence the input scheme:

  - ut [128, n_per]: shipped fully padded (small, 0.5 MB)
  - vt columns 0-1023: one fully padded [128, 1024] load -- only two
    semaphores gate the first matmul, earliest pipeline start
  - vt columns 1024+: rows 0-63 as 64-partition sprayed loads, rows
    64-66 on the slow path (84 KB), rows 67-127 zeroed on-chip by a
    chunked DVE memset (bitcast to f32; memset rejects float32r).
    This skips 1.6 MB/core of zero-row HBM traffic.

Output is staged into [128, 8192] SBUF strips; each [128, 2048] chunk
is stored with its own 1 MB DMA as soon as its exp completes, keeping
all 16 SDMA engines continuously fed (the kernel is output-write bound
at 32 MB/core; bursty store patterns measurably worsen cross-core HBM
stack contention).
"""

import numpy as np

N_X, N_Y, D = 8192, 8192, 64
N_CORES = 8
N_PER = N_X // N_CORES  # rows of x per core
K_LIVE = D + 3  # 67: [-2x, x2-2D, 1, 1] . [y, 1, y2-2D, 2D]
KP = 128  # padded partition dim for fast (16-engine) DMA loads

# Filled by kernel() with the BassKernelResults of the last run
# (test.py reads exec_time_ns from here when BASS_TRACE=1).
LAST_RESULTS = None

_BUILD_CACHE = {}


def _build(gamma: float, n_per: int, m_tot: int):
    """Build + compile the single-core Bass program (same on all cores)."""
    import concourse.bacc as bacc
    import concourse.mybir as mybir
    import concourse.tile as tile

    key = (gamma, n_per, m_tot)
    if key in _BUILD_CACHE:
        return _BUILD_CACHE[key]

    dt = mybir.dt
    nc = bacc.Bacc("TRN2", target_bir_lowering=False, debug=False)
    C0 = 1024  # first column block ships fully padded for a 2-sem fast start
    ut_d = nc.dram_tensor("ut", [KP, n_per], dt.float32r, kind="ExternalInput").ap()
    vt0_d = nc.dram_tensor("vt0", [KP, C0], dt.float32r, kind="ExternalInput").ap()
    vtr_d = nc.dram_tensor(
        "vtr", [K_LIVE, m_tot - C0], dt.float32r, kind="ExternalInput"
    ).ap()
    out_d = nc.dram_tensor("out", [n_per, m_tot], dt.float32, kind="ExternalOutput").ap()

    MB = n_per // 128  # output strips per core
    CHUNK = 2048  # ACT granularity: 4 PSUM banks per activation op
    NCHUNK = m_tot // CHUNK
    JB = CHUNK // 512  # matmuls (PSUM banks) per chunk

    with tile.TileContext(nc) as tc:
        with (
            tc.tile_pool(name="const", bufs=1) as cpool,
            tc.tile_pool(name="psum", bufs=2, space="PSUM") as psum_pool,
            tc.tile_pool(name="strip", bufs=3) as strip_pool,
        ):
            ut_s = cpool.tile([KP, n_per], dt.float32r, tag="ut")
            nc.sync.dma_start(ut_s[:], ut_d[:])
            # per-partition bias tile holding the centered-norm 2D shift
            # (arbitrary floats have no pre-baked const AP)
            bias_s = cpool.tile([128, 1], dt.float32, tag="bias")
            nc.vector.memset(bias_s[:], -gamma * 2.0 * float(D))
            # vt is loaded without its 61 zero-pad rows (1.9 MB of HBM
            # traffic): rows 0-63 spray across all 16 engines, rows 64-66
            # ride the slow single-engine path (96 KB), and rows 67-127 are
            # zeroed on-chip (bitcast to f32: memset rejects float32r).
            # Graded column chunks: small first chunk so the first matmul /
            # exp / store chain starts as early as possible.
            # columns 0-1023 arrive fully padded (one 2-sem 128-partition
            # load: earliest possible matmul start); the remaining columns
            # skip the 61 zero-pad rows (1.6 MB less HBM traffic): rows 0-63
            # spray across all 16 engines, rows 64-66 ride the slow
            # single-engine path, rows 67-127 are zeroed on-chip (bitcast to
            # f32: memset rejects float32r).  memset / aug / bulk are chunked
            # with the same grading so each column block completes ASAP.
            vt_s = cpool.tile([KP, m_tot], dt.float32r, tag="vt")
            # all memsets are hoisted before any dma_start: the matmuls pick
            # up a conservative dependency on them (the bitcast AP defeats
            # per-region tracking), so they must finish as early as possible
            # -- interleaved with the load issues the last one was measured
            # to start only at ~11.5 us and stall the first matmul to ~15 us
            # chunked loads (merging them into fewer, larger dmas was
            # measured to open a 5 us ramp bubble before chunk 1; the ~3 us
            # lag between load-bytes-landed and first matmul is invariant --
            # DMA completion-receipt latency -- so layout tweaks can't
            # remove it)
            # of the 3 augmentation rows only y2 (row 65) is real data; the
            # ones row (64) is memset and the 2D-constant row (66) is folded
            # into the activation's scalar bias.  Partial-partition
            # descriptors all pile onto one DMA engine, so cutting the aug
            # transfers from 9 descriptors (84 KB) to 3 (28 KB) trims the
            # busiest engine's queue.
            pos = C0
            for ncols in (1024, 2048, 4096):
                csl = slice(pos, pos + ncols)
                nc.vector.memset(vt_s[D:KP, csl].bitcast(dt.float32), 0.0)
                nc.vector.memset(vt_s[D : D + 1, csl].bitcast(dt.float32), 1.0)
                pos += ncols
            nc.sync.dma_start(vt_s[:, 0:C0], vt0_d[:])
            pos = C0
            for ncols in (1024, 2048, 4096):
                csl = slice(pos, pos + ncols)
                rsl = slice(pos - C0, pos - C0 + ncols)
                nc.sync.dma_start(
                    vt_s[D + 1 : D + 2, csl], vtr_d[D + 1 : D + 2, rsl]
                )
                nc.sync.dma_start(vt_s[:D, csl], vtr_d[:D, rsl])
                pos += ncols

            for m in range(MB):
                strip = strip_pool.tile([128, m_tot], dt.float32)
                msl = slice(m * 128, (m + 1) * 128)
                for c in range(NCHUNK):
                    csl = slice(c * CHUNK, (c + 1) * CHUNK)
                    ps = psum_pool.tile([128, CHUNK], dt.float32)
                    for j in range(JB):
                        jsl = slice(j * 512, (j + 1) * 512)
                        vsl = slice(c * CHUNK + j * 512, c * CHUNK + (j + 1) * 512)
                        nc.tensor.matmul(ps[:, jsl], ut_s[:, msl], vt_s[:, vsl])
                    nc.scalar.activation(
                        strip[:, csl],
                        ps[:],
                        mybir.ActivationFunctionType.Exp,
                        # the centered-norm 2D shift rides in the bias now
                        # (vt row 66 is zero; its old value 2D was folded out
                        # so the aug load is a single y2 row)
                        bias=bias_s[:],
                        scale=-gamma,
                    )
                    # mixed store granularity: strip 0 stores each 1 MB chunk
                    # immediately (keeps the pipeline ramp identical -- the
                    # uniform-2MB variant lost ~1 us here), later strips use
                    # 2 MB stores whose 16 KB descriptors halve the
                    # per-descriptor overhead on the engines.  4 MB per-strip
                    # stores measurably worsen cross-core HBM contention.
                    if m == 0:
                        nc.sync.dma_start(out_d[msl, csl], strip[:, csl])
                    elif c % 2 == 1:
                        osl = slice((c - 1) * CHUNK, (c + 1) * CHUNK)
                        nc.sync.dma_start(out_d[msl, osl], strip[:, osl])

    nc.compile()
    _BUILD_CACHE[key] = nc
    return nc


def _augment(x: np.ndarray, y: np.ndarray):
    """Host-side prep: transposed augmented operands, zero-padded to KP
    partitions (O(N*D) work)."""
    x = np.asarray(x, dtype=np.float32)
    y = np.asarray(y, dtype=np.float32)
    x2 = np.einsum("nd,nd->n", x, x).astype(np.float32)
    y2 = np.einsum("nd,nd->n", y, y).astype(np.float32)

    # Center the squared norms around their mean (E|x|^2 = D for unit-normal
    # data): the matmul then produces dist2 - 2D with small-magnitude
    # operands (better for the reduced-precision f32r path), and exp()'s
    # bias adds the -gamma*2D shift back.
    ut = np.zeros((KP, x.shape[0]), dtype=np.float32)
    ut[:D] = (-2.0 * x).T
    ut[D] = x2 - float(D)
    ut[D + 1] = 1.0
    ut[D + 2] = 1.0

    vt = np.empty((K_LIVE, y.shape[0]), dtype=np.float32)
    vt[:D] = y.T
    vt[D] = 1.0
    vt[D + 1] = y2 - float(D)
    vt[D + 2] = 2.0 * float(D)

    C0 = 1024
    vt0 = np.zeros((KP, C0), dtype=np.float32)
    vt0[:K_LIVE] = vt[:, :C0]
    vt0[D + 2] = 0.0  # 2D shift moved into the activation bias
    vtr = np.ascontiguousarray(vt[:, C0:])
    return ut, vt0, vtr


def kernel(x: np.ndarray, y: np.ndarray, gamma: np.ndarray) -> np.ndarray:
    global LAST_RESULTS
    from concourse.bass_utils import run_bass_kernel_spmd

    gamma_f = float(np.asarray(gamma).reshape(()))
    ut, vt0, vtr = _augment(x, y)

    nc = _build(gamma_f, N_PER, N_Y)

    in_maps = []
    for c in range(N_CORES):
        in_maps.append(
            {
                "ut": np.ascontiguousarray(ut[:, c * N_PER : (c + 1) * N_PER]),
                "vt0": vt0,
                "vtr": vtr,
            }
        )

    res = run_bass_kernel_spmd(nc, in_maps, core_ids=list(range(N_CORES)))
    LAST_RESULTS = res
    return np.concatenate([res.results[c]["out"] for c in range(N_CORES)], axis=0)
